# revision 4
# baseline (speedup 1.0000x reference)
"""Chamfer loss (ChamferDistanceL1-style) Trainium2 Bass kernel.

Problem: B=4 samples, N=M=4096 points, 3D. loss = mean_b 0.5*(m1_b + m2_b)
  m1 = masked mean over valid pred points of sqrt(min_m d[n,m])
  m2 = mean over target points of sqrt(min over *valid* n of d[n,m])
  d[n,m] = max(|p_n|^2 + |t_m|^2 - 2 p.t, 0)

Strategy (8 NeuronCores):
  - Host compacts each sample's pred points to the valid (label==1) subset
    (~halves the work), splits them across 2 cores -> 8 cores = 4 samples x 2.
  - Distances are produced by a single K=5 fp32 matmul per tile:
      lhsT col n = [-2px, -2py, -2pz, 1, |p_n|^2 (+BIG if padding)]
      rhs  col m = [ tx,   ty,   tz,  |t_m|^2, 1]
    so PSUM holds d[n,m] directly (before the max(.,0) clamp).
  - Per PSUM chunk [128, 2048] the DVE does a free-axis reduce-min (row mins)
    and a tensor_tensor min into a column accumulator [128, M].
  - Partition log-tree (128->64->32) on the column accumulator; host finishes
    the final 32-way min, clamp, sqrt, and means (tiny).
"""

import numpy as np

import concourse.bacc as bacc
import concourse.tile as tile
from concourse import mybir
from concourse.bass_utils import run_bass_kernel_spmd

F32 = mybir.dt.float32
BIG = np.float32(1e10)  # matches the reference's masking constant
_NC_CACHE = {}

# Fixed problem geometry (from the task spec); shapes are still derived from
# the actual inputs at call time, these are just tiling constants.
_P = 128          # partitions / rows per weight tile
_MM_FREE = 512    # fp32 matmul moving-dim limit (one PSUM bank)
_CHUNK = 2048     # PSUM chunk (4 banks); 2 bufs = all 8 banks


def _build_nc(r_tiles: int, m_pad: int):
    """Build + finalize the per-core Bass program for R=128*r_tiles pred rows
    and m_pad (multiple of _CHUNK) target columns."""
    R = r_tiles * _P
    n_chunks = m_pad // _CHUNK

    nc = bacc.Bacc("TRN2", target_bir_lowering=False)
    inp = nc.dram_tensor("inp", [5, R + m_pad], F32, kind="ExternalInput")
    rowmin_d = nc.dram_tensor("rowmin", [_P, r_tiles], F32, kind="ExternalOutput")
    colmin_d = nc.dram_tensor("colmin128", [_P, m_pad], F32, kind="ExternalOutput")

    with tile.TileContext(nc) as tc:
        with tc.tile_pool(name="io", bufs=1) as io, \
             tc.tile_pool(name="ps", bufs=2, space="PSUM") as psp:
            in_sb = io.tile([5, R + m_pad], F32)
            nc.sync.dma_start(out=in_sb[:], in_=inp[:, :])

            colacc = io.tile([_P, m_pad], F32)
            nc.any.memset(colacc[:], 1e30)

            rowstage = io.tile([_P, r_tiles * n_chunks], F32)

            for i in range(r_tiles):
                lhsT = in_sb[:, i * _P:(i + 1) * _P]
                for c in range(n_chunks):
                    ps = psp.tile([_P, _CHUNK], F32, tag="ps")
                    for s in range(_CHUNK // _MM_FREE):
                        col0 = R + c * _CHUNK + s * _MM_FREE
                        nc.tensor.matmul(
                            ps[:, s * _MM_FREE:(s + 1) * _MM_FREE],
                            lhsT,
                            in_sb[:, col0:col0 + _MM_FREE],
                            start=True, stop=True,
                        )
                    k = i * n_chunks + c
                    nc.vector.tensor_reduce(
                        rowstage[:, k:k + 1], ps[:],
                        axis=mybir.AxisListType.X, op=mybir.AluOpType.min,
                    )
                    nc.vector.tensor_tensor(
                        out=colacc[:, c * _CHUNK:(c + 1) * _CHUNK],
                        in0=ps[:],
                        in1=colacc[:, c * _CHUNK:(c + 1) * _CHUNK],
                        op=mybir.AluOpType.min,
                    )

            rowmin_sb = io.tile([_P, r_tiles], F32)
            nc.vector.tensor_reduce(
                rowmin_sb[:],
                rowstage[:].rearrange("p (i c) -> p i c", c=n_chunks),
                axis=mybir.AxisListType.X, op=mybir.AluOpType.min,
            )
            nc.sync.dma_start(out=rowmin_d[:, :], in_=rowmin_sb[:])

            # Host finishes the 128-way partition min (cross-partition
            # tensor ops are illegal on SBUF; DMA out is cheap enough).
            nc.sync.dma_start(out=colmin_d[:, :], in_=colacc[:, :])
    nc.finalize()
    return nc


def _get_nc(r_tiles: int, m_pad: int):
    key = (r_tiles, m_pad)
    if key not in _NC_CACHE:
        _NC_CACHE[key] = _build_nc(r_tiles, m_pad)
    return _NC_CACHE[key]


def _chamfer_numpy(p, t, mask):
    """Blocked numpy fallback (exact), for odd configurations."""
    B = p.shape[0]
    per_sample = np.zeros(B, dtype=np.float64)
    for b in range(B):
        pb, tb = p[b], t[b]
        tn = (tb * tb).sum(1)
        pn = (pb * pb).sum(1)
        rowmin = np.full(pb.shape[0], np.inf, dtype=np.float32)
        colmin = np.full(tb.shape[0], np.float32(BIG), dtype=np.float32)
        step = 512
        for i in range(0, pb.shape[0], step):
            d = (pn[i:i + step, None] + tn[None, :]
                 - 2.0 * (pb[i:i + step] @ tb.T)).astype(np.float32)
            d = np.maximum(d, 0.0)
            rowmin[i:i + step] = d.min(axis=1)
            mrows = mask[b, i:i + step]
            if mrows.any():
                colmin = np.minimum(colmin, d[mrows].min(axis=0))
        cnt = max(int(mask[b].sum()), 1)
        m1 = np.sqrt(rowmin[mask[b]]).sum() / cnt
        m2 = np.sqrt(colmin).mean()
        per_sample[b] = 0.5 * (m1 + m2)
    return np.float32(per_sample.mean())


def kernel(pred_pc, target, label, nums, dense_nums):
    B = int(np.asarray(nums).shape[0])
    p = np.ascontiguousarray(np.asarray(pred_pc, dtype=np.float32)).reshape(B, -1, 3)
    t = np.ascontiguousarray(np.asarray(target, dtype=np.float32)).reshape(B, -1, 3)
    N = p.shape[1]
    M = t.shape[1]
    mask = (np.asarray(label).reshape(B, N) == 1)

    if B < 1 or B > 8 or M < 1:
        return _chamfer_numpy(p, t, mask)

    cps = max(1, 8 // B)          # cores per sample
    n_cores = B * cps
    m_pad = ((M + _CHUNK - 1) // _CHUNK) * _CHUNK

    # Split each sample's valid pred points across its cores.
    parts = []                    # (sample, pts[r,3]) per core
    for b in range(B):
        pv = p[b][mask[b]]
        for chunk in np.array_split(pv, cps, axis=0):
            parts.append((b, np.ascontiguousarray(chunk)))
    rmax = max(c.shape[0] for _, c in parts)
    R = max(_P, ((rmax + _P - 1) // _P) * _P)
    r_tiles = R // _P

    nc = _get_nc(r_tiles, m_pad)

    in_maps = []
    for b, pts in parts:
        r = pts.shape[0]
        inp = np.zeros((5, R + m_pad), dtype=np.float32)
        if r > 0:
            inp[0:3, :r] = -2.0 * pts.T
            inp[4, :r] = (pts * pts).sum(1)
        inp[3, :R] = 1.0
        inp[4, r:R] = BIG
        inp[0:3, R:R + M] = t[b].T
        inp[3, R:R + M] = (t[b] * t[b]).sum(1)
        if m_pad > M:               # padding cols must never win a row-min
            inp[3, R + M:] = BIG
        inp[4, R:] = 1.0
        in_maps.append({"inp": inp})

    res = run_bass_kernel_spmd(nc, in_maps, core_ids=list(range(n_cores)))

    per_sample = np.zeros(B, dtype=np.float64)
    for b in range(B):
        d1_sum = 0.0
        colmin = np.full(M, np.float32(BIG), dtype=np.float32)
        for h in range(cps):
            core = b * cps + h
            r = parts[core][1].shape[0]
            out = res.results[core]
            if r > 0:
                rowmin = out["rowmin"].T.ravel()[:r]       # n = i*128 + p
                d1_sum += np.sqrt(np.maximum(rowmin, 0.0)).sum(dtype=np.float64)
            colmin = np.minimum(colmin, out["colmin128"].min(axis=0)[:M])
        cnt = max(int(mask[b].sum()), 1)
        m1 = d1_sum / cnt
        m2 = np.sqrt(np.maximum(colmin, 0.0)).mean(dtype=np.float64)
        per_sample[b] = 0.5 * (m1 + m2)

    return np.float32(per_sample.mean())


# revision 5
# speedup vs baseline: 1.0290x; 1.0290x over previous
"""Chamfer loss (ChamferDistanceL1-style) Trainium2 Bass kernel.

Problem: B=4 samples, N=M=4096 points, 3D. loss = mean_b 0.5*(m1_b + m2_b)
  m1 = masked mean over valid pred points of sqrt(min_m d[n,m])
  m2 = mean over target points of sqrt(min over *valid* n of d[n,m])
  d[n,m] = max(|p_n|^2 + |t_m|^2 - 2 p.t, 0)

Strategy (8 NeuronCores):
  - Host compacts each sample's pred points to the valid (label==1) subset
    (~halves the work), splits them across 2 cores -> 8 cores = 4 samples x 2.
  - Distances are produced by a single K=5 fp32 matmul per tile:
      lhsT col n = [-2px, -2py, -2pz, 1, |p_n|^2 (+BIG if padding)]
      rhs  col m = [ tx,   ty,   tz,  |t_m|^2, 1]
    so PSUM holds d[n,m] directly (before the max(.,0) clamp).
  - Per PSUM chunk [128, 2048] the DVE does a free-axis reduce-min (row mins)
    and a fused (x * -1) max accumulate into a negated column accumulator.
  - GPSIMD finishes each chunk's 128-way partition max (overlapped with the
    next chunk's compute); host does the final clamp/sqrt/means (tiny).
"""

import numpy as np

import concourse.bacc as bacc
import concourse.bass_isa as bass_isa
import concourse.tile as tile
from concourse import mybir
from concourse.bass_utils import run_bass_kernel_spmd

F32 = mybir.dt.float32
BIG = np.float32(1e10)  # matches the reference's masking constant
_NC_CACHE = {}

_P = 128          # partitions / rows per weight tile
_MM_FREE = 512    # fp32 matmul moving-dim limit (one PSUM bank)
_CHUNK = 2048     # PSUM chunk (4 banks); 2 bufs = all 8 banks


def _build_nc(r_tiles: int, m_pad: int):
    """Build + finalize the per-core Bass program for R=128*r_tiles pred rows
    and m_pad (multiple of _CHUNK) target columns."""
    R = r_tiles * _P
    n_chunks = m_pad // _CHUNK

    nc = bacc.Bacc("TRN2", target_bir_lowering=False)
    inp = nc.dram_tensor("inp", [5, R + m_pad], F32, kind="ExternalInput")
    rowmin_d = nc.dram_tensor("rowmin", [_P, r_tiles], F32, kind="ExternalOutput")
    colmax_d = nc.dram_tensor("colmax", [1, m_pad], F32, kind="ExternalOutput")

    with tile.TileContext(nc) as tc:
        with tc.tile_pool(name="io", bufs=1) as io, \
             tc.tile_pool(name="ps", bufs=2, space="PSUM") as psp:
            in_sb = io.tile([5, R + m_pad], F32)
            nc.sync.dma_start(out=in_sb[:], in_=inp[:, :])

            # negated column accumulator: holds max(-d) = -min(d)
            colacc = io.tile([_P, m_pad], F32)
            nc.any.memset(colacc[:], -3e38)
            colred = io.tile([_P, m_pad], F32)

            rowstage = io.tile([_P, r_tiles * n_chunks], F32)

            for c in range(n_chunks):
                for i in range(r_tiles):
                    lhsT = in_sb[:, i * _P:(i + 1) * _P]
                    ps = psp.tile([_P, _CHUNK], F32, tag="ps")
                    for s in range(_CHUNK // _MM_FREE):
                        col0 = R + c * _CHUNK + s * _MM_FREE
                        nc.tensor.matmul(
                            ps[:, s * _MM_FREE:(s + 1) * _MM_FREE],
                            lhsT,
                            in_sb[:, col0:col0 + _MM_FREE],
                            start=True, stop=True,
                        )
                    k = i * n_chunks + c
                    nc.vector.tensor_reduce(
                        rowstage[:, k:k + 1], ps[:],
                        axis=mybir.AxisListType.X, op=mybir.AluOpType.min,
                    )
                    cs = slice(c * _CHUNK, (c + 1) * _CHUNK)
                    nc.vector.scalar_tensor_tensor(
                        out=colacc[:, cs],
                        in0=ps[:],
                        scalar=-1.0,
                        in1=colacc[:, cs],
                        op0=mybir.AluOpType.mult,
                        op1=mybir.AluOpType.max,
                    )
                # chunk done: 128-way partition max on GPSIMD (overlaps the
                # next chunk's matmuls/DVE work)
                cs = slice(c * _CHUNK, (c + 1) * _CHUNK)
                nc.gpsimd.partition_all_reduce(
                    colred[:, cs], colacc[:, cs],
                    channels=_P, reduce_op=bass_isa.ReduceOp.max,
                )
                nc.sync.dma_start(out=colmax_d[:, cs], in_=colred[0:1, cs])

            rowmin_sb = io.tile([_P, r_tiles], F32)
            nc.vector.tensor_reduce(
                rowmin_sb[:],
                rowstage[:].rearrange("p (i c) -> p i c", c=n_chunks),
                axis=mybir.AxisListType.X, op=mybir.AluOpType.min,
            )
            nc.sync.dma_start(out=rowmin_d[:, :], in_=rowmin_sb[:])
    nc.finalize()
    return nc


def _get_nc(r_tiles: int, m_pad: int):
    key = (r_tiles, m_pad)
    if key not in _NC_CACHE:
        _NC_CACHE[key] = _build_nc(r_tiles, m_pad)
    return _NC_CACHE[key]


def _chamfer_numpy(p, t, mask):
    """Blocked numpy fallback (exact), for odd configurations."""
    B = p.shape[0]
    per_sample = np.zeros(B, dtype=np.float64)
    for b in range(B):
        pb, tb = p[b], t[b]
        tn = (tb * tb).sum(1)
        pn = (pb * pb).sum(1)
        rowmin = np.full(pb.shape[0], np.inf, dtype=np.float32)
        colmin = np.full(tb.shape[0], np.float32(BIG), dtype=np.float32)
        step = 512
        for i in range(0, pb.shape[0], step):
            d = (pn[i:i + step, None] + tn[None, :]
                 - 2.0 * (pb[i:i + step] @ tb.T)).astype(np.float32)
            d = np.maximum(d, 0.0)
            rowmin[i:i + step] = d.min(axis=1)
            mrows = mask[b, i:i + step]
            if mrows.any():
                colmin = np.minimum(colmin, d[mrows].min(axis=0))
        cnt = max(int(mask[b].sum()), 1)
        m1 = np.sqrt(rowmin[mask[b]]).sum() / cnt
        m2 = np.sqrt(colmin).mean()
        per_sample[b] = 0.5 * (m1 + m2)
    return np.float32(per_sample.mean())


def kernel(pred_pc, target, label, nums, dense_nums):
    B = int(np.asarray(nums).shape[0])
    p = np.ascontiguousarray(np.asarray(pred_pc, dtype=np.float32)).reshape(B, -1, 3)
    t = np.ascontiguousarray(np.asarray(target, dtype=np.float32)).reshape(B, -1, 3)
    N = p.shape[1]
    M = t.shape[1]
    mask = (np.asarray(label).reshape(B, N) == 1)

    if B < 1 or B > 8 or M < 1:
        return _chamfer_numpy(p, t, mask)

    cps = max(1, 8 // B)          # cores per sample
    n_cores = B * cps
    m_pad = ((M + _CHUNK - 1) // _CHUNK) * _CHUNK

    # Split each sample's valid pred points across its cores.
    parts = []                    # (sample, pts[r,3]) per core
    for b in range(B):
        pv = p[b][mask[b]]
        for chunk in np.array_split(pv, cps, axis=0):
            parts.append((b, np.ascontiguousarray(chunk)))
    rmax = max(c.shape[0] for _, c in parts)
    R = max(_P, ((rmax + _P - 1) // _P) * _P)
    r_tiles = R // _P

    nc = _get_nc(r_tiles, m_pad)

    in_maps = []
    for b, pts in parts:
        r = pts.shape[0]
        inp = np.zeros((5, R + m_pad), dtype=np.float32)
        if r > 0:
            inp[0:3, :r] = -2.0 * pts.T
            inp[4, :r] = (pts * pts).sum(1)
        inp[3, :R] = 1.0
        inp[4, r:R] = BIG
        inp[0:3, R:R + M] = t[b].T
        inp[3, R:R + M] = (t[b] * t[b]).sum(1)
        if m_pad > M:               # padding cols must never win a row-min
            inp[3, R + M:] = BIG
        inp[4, R:] = 1.0
        in_maps.append({"inp": inp})

    res = run_bass_kernel_spmd(nc, in_maps, core_ids=list(range(n_cores)))

    per_sample = np.zeros(B, dtype=np.float64)
    for b in range(B):
        d1_sum = 0.0
        colmin = np.full(M, np.float32(BIG), dtype=np.float32)
        for h in range(cps):
            core = b * cps + h
            r = parts[core][1].shape[0]
            out = res.results[core]
            if r > 0:
                rowmin = out["rowmin"].T.ravel()[:r]       # n = i*128 + p
                d1_sum += np.sqrt(np.maximum(rowmin, 0.0)).sum(dtype=np.float64)
            colmin = np.minimum(colmin, -out["colmax"][0, :M])
        cnt = max(int(mask[b].sum()), 1)
        m1 = d1_sum / cnt
        m2 = np.sqrt(np.maximum(colmin, 0.0)).mean(dtype=np.float64)
        per_sample[b] = 0.5 * (m1 + m2)

    return np.float32(per_sample.mean())


# revision 9
# speedup vs baseline: 1.0880x; 1.0573x over previous
"""Chamfer loss (ChamferDistanceL1-style) Trainium2 Bass kernel.

Problem: B=4 samples, N=M=4096 points, 3D. loss = mean_b 0.5*(m1_b + m2_b)
  m1 = masked mean over valid pred points of sqrt(min_m d[n,m])
  m2 = mean over target points of sqrt(min over *valid* n of d[n,m])
  d[n,m] = max(|p_n|^2 + |t_m|^2 - 2 p.t, 0)

Strategy (8 NeuronCores):
  - Host compacts each sample's pred points to the valid (label==1) subset
    (~halves the work), splits them across 2 cores -> 8 cores = 4 samples x 2.
  - Distances are produced by a single K=5 fp32 matmul per tile:
      lhsT col n = [-2px, -2py, -2pz, 1, |p_n|^2 (+BIG if padding)]
      rhs  col m = [ tx,   ty,   tz,  |t_m|^2, 1]
    so PSUM holds d[n,m] directly (before the max(.,0) clamp).
  - Per PSUM chunk [128, 2048]:
      DVE: free-axis reduce-min (exact fp32 row mins)
      ACT: negated fp16 copy PSUM -> SBUF (idle engine, off critical path)
      DVE: fp16 tensor_tensor max into the negated column accumulator
           (fp16 SBUF step-1 -> 2x DVE mode, halves the column-pass cost)
  - GPSIMD finishes each chunk's 128-way partition max (overlapped with the
    next chunk's compute); host does the final clamp/sqrt/means (tiny).
  - fp16 only touches the d2 (target->pred) path as a value rounding of the
    already-exact fp32 distances: |err| <= 2^-11 relative, ~1e-4 on the loss.
"""

import numpy as np

import concourse.bacc as bacc
import concourse.bass_isa as bass_isa
import concourse.tile as tile
from concourse import mybir
from concourse.bass_utils import run_bass_kernel_spmd

F32 = mybir.dt.float32
F16 = mybir.dt.float16
BIG = np.float32(1e10)  # matches the reference's masking constant
_NC_CACHE = {}

_P = 128          # partitions / rows per weight tile
_MM_FREE = 512    # fp32 matmul moving-dim limit (one PSUM bank)
_CHUNK = 2048     # PSUM chunk (4 banks); 2 bufs = all 8 banks


def _build_nc(r_tiles: int, m_pad: int):
    """Build + finalize the per-core Bass program for R=128*r_tiles pred rows
    and m_pad (multiple of _CHUNK) target columns."""
    R = r_tiles * _P
    n_chunks = m_pad // _CHUNK

    nc = bacc.Bacc("TRN2", target_bir_lowering=False)
    inp = nc.dram_tensor("inp", [5, R + m_pad], F32, kind="ExternalInput")
    rowmin_d = nc.dram_tensor("rowmin", [_P, r_tiles], F32, kind="ExternalOutput")
    colmax_d = nc.dram_tensor("colmax", [1, m_pad], F32, kind="ExternalOutput")

    with tile.TileContext(nc) as tc:
        with tc.tile_pool(name="io", bufs=1) as io, \
             tc.tile_pool(name="ps", bufs=2, space="PSUM") as psp:
            in_sb = io.tile([5, R + m_pad], F32)
            nc.sync.dma_start(out=in_sb[:], in_=inp[:, :])

            # negated fp16 column accumulator: holds max(-d) = -min(d)
            colacc = io.tile([_P, m_pad], F16)
            nc.any.memset(colacc[:], -60000.0)
            colred = io.tile([_P, m_pad], F32)

            rowstage = io.tile([_P, r_tiles * n_chunks], F32)

            with tc.tile_pool(name="scr", bufs=3) as scrp:
                for c in range(n_chunks):
                    for i in range(r_tiles):
                        lhsT = in_sb[:, i * _P:(i + 1) * _P]
                        ps = psp.tile([_P, _CHUNK], F32, tag="ps")
                        for s in range(_CHUNK // _MM_FREE):
                            col0 = R + c * _CHUNK + s * _MM_FREE
                            nc.tensor.matmul(
                                ps[:, s * _MM_FREE:(s + 1) * _MM_FREE],
                                lhsT,
                                in_sb[:, col0:col0 + _MM_FREE],
                                start=True, stop=True,
                            )
                        k = i * n_chunks + c
                        nc.vector.tensor_reduce(
                            rowstage[:, k:k + 1], ps[:],
                            axis=mybir.AxisListType.X, op=mybir.AluOpType.min,
                        )
                        # ACT: scr = -d in fp16 (off the DVE critical path)
                        scr = scrp.tile([_P, _CHUNK], F16, tag="scr")
                        nc.scalar.mul(scr[:], ps[:], -1.0)
                        cs = slice(c * _CHUNK, (c + 1) * _CHUNK)
                        nc.vector.tensor_tensor(
                            out=colacc[:, cs], in0=scr[:], in1=colacc[:, cs],
                            op=mybir.AluOpType.max,
                        )
                    # chunk done: 128-way partition max on GPSIMD (overlaps
                    # the next chunk's matmuls/DVE work)
                    cs = slice(c * _CHUNK, (c + 1) * _CHUNK)
                    nc.gpsimd.partition_all_reduce(
                        colred[:, cs], colacc[:, cs],
                        channels=_P, reduce_op=bass_isa.ReduceOp.max,
                    )
                    nc.sync.dma_start(out=colmax_d[:, cs], in_=colred[0:1, cs])

            rowmin_sb = io.tile([_P, r_tiles], F32)
            nc.vector.tensor_reduce(
                rowmin_sb[:],
                rowstage[:].rearrange("p (i c) -> p i c", c=n_chunks),
                axis=mybir.AxisListType.X, op=mybir.AluOpType.min,
            )
            nc.sync.dma_start(out=rowmin_d[:, :], in_=rowmin_sb[:])
    nc.finalize()
    return nc


def _get_nc(r_tiles: int, m_pad: int):
    key = (r_tiles, m_pad)
    if key not in _NC_CACHE:
        _NC_CACHE[key] = _build_nc(r_tiles, m_pad)
    return _NC_CACHE[key]


def _chamfer_numpy(p, t, mask):
    """Blocked numpy fallback (exact), for odd configurations."""
    B = p.shape[0]
    per_sample = np.zeros(B, dtype=np.float64)
    for b in range(B):
        pb, tb = p[b], t[b]
        tn = (tb * tb).sum(1)
        pn = (pb * pb).sum(1)
        rowmin = np.full(pb.shape[0], np.inf, dtype=np.float32)
        colmin = np.full(tb.shape[0], np.float32(BIG), dtype=np.float32)
        step = 512
        for i in range(0, pb.shape[0], step):
            d = (pn[i:i + step, None] + tn[None, :]
                 - 2.0 * (pb[i:i + step] @ tb.T)).astype(np.float32)
            d = np.maximum(d, 0.0)
            rowmin[i:i + step] = d.min(axis=1)
            mrows = mask[b, i:i + step]
            if mrows.any():
                colmin = np.minimum(colmin, d[mrows].min(axis=0))
        cnt = max(int(mask[b].sum()), 1)
        m1 = np.sqrt(rowmin[mask[b]]).sum() / cnt
        m2 = np.sqrt(colmin).mean()
        per_sample[b] = 0.5 * (m1 + m2)
    return np.float32(per_sample.mean())


def kernel(pred_pc, target, label, nums, dense_nums):
    B = int(np.asarray(nums).shape[0])
    p = np.ascontiguousarray(np.asarray(pred_pc, dtype=np.float32)).reshape(B, -1, 3)
    t = np.ascontiguousarray(np.asarray(target, dtype=np.float32)).reshape(B, -1, 3)
    N = p.shape[1]
    M = t.shape[1]
    mask = (np.asarray(label).reshape(B, N) == 1)

    if B < 1 or B > 8 or M < 1:
        return _chamfer_numpy(p, t, mask)

    cps = max(1, 8 // B)          # cores per sample
    n_cores = B * cps
    m_pad = ((M + _CHUNK - 1) // _CHUNK) * _CHUNK

    # Split each sample's valid pred points across its cores.
    parts = []                    # (sample, pts[r,3]) per core
    for b in range(B):
        pv = p[b][mask[b]]
        for chunk in np.array_split(pv, cps, axis=0):
            parts.append((b, np.ascontiguousarray(chunk)))
    rmax = max(c.shape[0] for _, c in parts)
    R = max(_P, ((rmax + _P - 1) // _P) * _P)
    r_tiles = R // _P

    nc = _get_nc(r_tiles, m_pad)

    in_maps = []
    for b, pts in parts:
        r = pts.shape[0]
        inp = np.zeros((5, R + m_pad), dtype=np.float32)
        if r > 0:
            inp[0:3, :r] = -2.0 * pts.T
            inp[4, :r] = (pts * pts).sum(1)
        inp[3, :R] = 1.0
        inp[4, r:R] = BIG
        inp[0:3, R:R + M] = t[b].T
        inp[3, R:R + M] = (t[b] * t[b]).sum(1)
        if m_pad > M:               # padding cols must never win a row-min
            inp[3, R + M:] = BIG
        inp[4, R:] = 1.0
        in_maps.append({"inp": inp})

    res = run_bass_kernel_spmd(nc, in_maps, core_ids=list(range(n_cores)))

    per_sample = np.zeros(B, dtype=np.float64)
    for b in range(B):
        d1_sum = 0.0
        colmin = np.full(M, np.float32(BIG), dtype=np.float32)
        for h in range(cps):
            core = b * cps + h
            r = parts[core][1].shape[0]
            out = res.results[core]
            if r > 0:
                rowmin = out["rowmin"].T.ravel()[:r]       # n = i*128 + p
                d1_sum += np.sqrt(np.maximum(rowmin, 0.0)).sum(dtype=np.float64)
            colmin = np.minimum(colmin, -out["colmax"][0, :M])
        nv = int(mask[b].sum())
        cnt = max(nv, 1)
        m1 = d1_sum / cnt
        if nv == 0:
            colmin[:] = BIG        # reference: all rows masked -> d = BIG
        m2 = np.sqrt(np.maximum(colmin, 0.0)).mean(dtype=np.float64)
        per_sample[b] = 0.5 * (m1 + m2)

    return np.float32(per_sample.mean())


# revision 12
# speedup vs baseline: 1.1099x; 1.0201x over previous
"""Chamfer loss (ChamferDistanceL1-style) Trainium2 Bass kernel.

Problem: B=4 samples, N=M=4096 points, 3D. loss = mean_b 0.5*(m1_b + m2_b)
  m1 = masked mean over valid pred points of sqrt(min_m d[n,m])
  m2 = mean over target points of sqrt(min over *valid* n of d[n,m])
  d[n,m] = max(|p_n|^2 + |t_m|^2 - 2 p.t, 0)

Strategy (8 NeuronCores):
  - Host compacts each sample's pred points to the valid (label==1) subset
    (~halves the work), splits them across 2 cores -> 8 cores = 4 samples x 2.
  - Distances are produced by a single K=5 fp32 matmul per tile:
      lhsT col n = [-2px, -2py, -2pz, 1, |p_n|^2 (+BIG if padding)]
      rhs  col m = [ tx,   ty,   tz,  |t_m|^2, 1]
    so PSUM holds d[n,m] directly (before the max(.,0) clamp).
  - Per PSUM chunk [128, 2048]:
      DVE: free-axis reduce-min (exact fp32 row mins)
      ACT: negated fp16 copy PSUM -> SBUF (idle engine, off critical path)
      DVE: fp16 tensor_tensor max into the negated column accumulator
           (fp16 SBUF step-1 -> 2x DVE mode, halves the column-pass cost)
  - GPSIMD finishes each chunk's 128-way partition max (overlapped with the
    next chunk's compute); host does the final clamp/sqrt/means (tiny).
  - fp16 only touches the d2 (target->pred) path as a value rounding of the
    already-exact fp32 distances: |err| <= 2^-11 relative, ~1e-4 on the loss.
"""

import numpy as np

import concourse.bacc as bacc
import concourse.bass_isa as bass_isa
import concourse.tile as tile
from concourse import mybir
from concourse.bass_utils import run_bass_kernel_spmd

F32 = mybir.dt.float32
F16 = mybir.dt.float16
BIG = np.float32(1e10)  # matches the reference's masking constant
_NC_CACHE = {}

_P = 128          # partitions / rows per weight tile
_MM_FREE = 512    # fp32 matmul moving-dim limit (one PSUM bank)
_CHUNK = 2048     # PSUM chunk (4 banks); 2 bufs = all 8 banks


def _build_nc(r_tiles: int, m_pad: int):
    """Build + finalize the per-core Bass program for R=128*r_tiles pred rows
    and m_pad (multiple of _CHUNK) target columns."""
    R = r_tiles * _P
    n_chunks = m_pad // _CHUNK

    nc = bacc.Bacc("TRN2", target_bir_lowering=False)
    inp = nc.dram_tensor("inp", [5, R + m_pad], F32, kind="ExternalInput")
    rowmin_d = nc.dram_tensor("rowmin", [_P, r_tiles], F32, kind="ExternalOutput")
    colmax_d = nc.dram_tensor("colmax", [1, m_pad], F32, kind="ExternalOutput")
    warm_d = nc.dram_tensor("warm", [_P, 1], F32, kind="ExternalOutput")

    with tile.TileContext(nc) as tc:
        with tc.tile_pool(name="io", bufs=1) as io, \
             tc.tile_pool(name="ps", bufs=2, space="PSUM") as psp:
            # PE warmup: ~5 dummy matmuls during the input DMA keep the HAM
            # clock-gate busy so real matmuls run at full clock from the start.
            wsrc = io.tile([5, _MM_FREE], F32)
            nc.vector.memset(wsrc[:], 0.0)
            wps = psp.tile([_P, _MM_FREE], F32, tag="ps")
            for _ in range(5):
                nc.tensor.matmul(wps[:], wsrc[:, 0:_P], wsrc[:],
                                 start=True, stop=True)
            warm_sb = io.tile([_P, 1], F32)
            nc.vector.tensor_reduce(warm_sb[:], wps[:],
                                    axis=mybir.AxisListType.X,
                                    op=mybir.AluOpType.max)
            nc.sync.dma_start(out=warm_d[:, :], in_=warm_sb[:])

            # weights first (small), then one DMA per rhs chunk: the first
            # matmuls only wait on their own chunk's DMA.
            in_sb = io.tile([5, R + m_pad], F32)
            nc.sync.dma_start(out=in_sb[:, :R], in_=inp[:, :R])
            for c in range(n_chunks):
                cs = slice(R + c * _CHUNK, R + (c + 1) * _CHUNK)
                nc.sync.dma_start(out=in_sb[:, cs], in_=inp[:, cs])

            # negated fp16 column accumulator: holds max(-d) = -min(d)
            colacc = io.tile([_P, m_pad], F16)
            nc.any.memset(colacc[:], -60000.0)
            colred = io.tile([_P, m_pad], F32)

            rowstage = io.tile([_P, r_tiles * n_chunks], F32)

            with tc.tile_pool(name="scr", bufs=3) as scrp:
                for c in range(n_chunks):
                    for i in range(r_tiles):
                        lhsT = in_sb[:, i * _P:(i + 1) * _P]
                        ps = psp.tile([_P, _CHUNK], F32, tag="ps")
                        for s in range(_CHUNK // _MM_FREE):
                            col0 = R + c * _CHUNK + s * _MM_FREE
                            nc.tensor.matmul(
                                ps[:, s * _MM_FREE:(s + 1) * _MM_FREE],
                                lhsT,
                                in_sb[:, col0:col0 + _MM_FREE],
                                start=True, stop=True,
                            )
                        # ACT: scr = -d in fp16 (off the DVE critical path)
                        scr = scrp.tile([_P, _CHUNK], F16, tag="scr")
                        nc.scalar.mul(scr[:], ps[:], -1.0)
                        k = i * n_chunks + c
                        nc.vector.tensor_reduce(
                            rowstage[:, k:k + 1], ps[:],
                            axis=mybir.AxisListType.X, op=mybir.AluOpType.min,
                        )
                        cs = slice(c * _CHUNK, (c + 1) * _CHUNK)
                        nc.vector.tensor_tensor(
                            out=colacc[:, cs], in0=scr[:], in1=colacc[:, cs],
                            op=mybir.AluOpType.max,
                        )
                    # chunk done: 128-way partition max on GPSIMD (overlaps
                    # the next chunk's matmuls/DVE work)
                    cs = slice(c * _CHUNK, (c + 1) * _CHUNK)
                    nc.gpsimd.partition_all_reduce(
                        colred[:, cs], colacc[:, cs],
                        channels=_P, reduce_op=bass_isa.ReduceOp.max,
                    )
                    nc.sync.dma_start(out=colmax_d[:, cs], in_=colred[0:1, cs])

            rowmin_sb = io.tile([_P, r_tiles], F32)
            nc.vector.tensor_reduce(
                rowmin_sb[:],
                rowstage[:].rearrange("p (i c) -> p i c", c=n_chunks),
                axis=mybir.AxisListType.X, op=mybir.AluOpType.min,
            )
            nc.sync.dma_start(out=rowmin_d[:, :], in_=rowmin_sb[:])
    nc.finalize()
    return nc


def _get_nc(r_tiles: int, m_pad: int):
    key = (r_tiles, m_pad)
    if key not in _NC_CACHE:
        _NC_CACHE[key] = _build_nc(r_tiles, m_pad)
    return _NC_CACHE[key]


def _chamfer_numpy(p, t, mask):
    """Blocked numpy fallback (exact), for odd configurations."""
    B = p.shape[0]
    per_sample = np.zeros(B, dtype=np.float64)
    for b in range(B):
        pb, tb = p[b], t[b]
        tn = (tb * tb).sum(1)
        pn = (pb * pb).sum(1)
        rowmin = np.full(pb.shape[0], np.inf, dtype=np.float32)
        colmin = np.full(tb.shape[0], np.float32(BIG), dtype=np.float32)
        step = 512
        for i in range(0, pb.shape[0], step):
            d = (pn[i:i + step, None] + tn[None, :]
                 - 2.0 * (pb[i:i + step] @ tb.T)).astype(np.float32)
            d = np.maximum(d, 0.0)
            rowmin[i:i + step] = d.min(axis=1)
            mrows = mask[b, i:i + step]
            if mrows.any():
                colmin = np.minimum(colmin, d[mrows].min(axis=0))
        cnt = max(int(mask[b].sum()), 1)
        m1 = np.sqrt(rowmin[mask[b]]).sum() / cnt
        m2 = np.sqrt(colmin).mean()
        per_sample[b] = 0.5 * (m1 + m2)
    return np.float32(per_sample.mean())


def kernel(pred_pc, target, label, nums, dense_nums):
    B = int(np.asarray(nums).shape[0])
    p = np.ascontiguousarray(np.asarray(pred_pc, dtype=np.float32)).reshape(B, -1, 3)
    t = np.ascontiguousarray(np.asarray(target, dtype=np.float32)).reshape(B, -1, 3)
    N = p.shape[1]
    M = t.shape[1]
    mask = (np.asarray(label).reshape(B, N) == 1)

    if B < 1 or B > 8 or M < 1:
        return _chamfer_numpy(p, t, mask)

    cps = max(1, 8 // B)          # cores per sample
    n_cores = B * cps
    m_pad = ((M + _CHUNK - 1) // _CHUNK) * _CHUNK

    # Split each sample's valid pred points across its cores.
    parts = []                    # (sample, pts[r,3]) per core
    for b in range(B):
        pv = p[b][mask[b]]
        for chunk in np.array_split(pv, cps, axis=0):
            parts.append((b, np.ascontiguousarray(chunk)))
    rmax = max(c.shape[0] for _, c in parts)
    R = max(_P, ((rmax + _P - 1) // _P) * _P)
    r_tiles = R // _P

    nc = _get_nc(r_tiles, m_pad)

    in_maps = []
    for b, pts in parts:
        r = pts.shape[0]
        inp = np.zeros((5, R + m_pad), dtype=np.float32)
        if r > 0:
            inp[0:3, :r] = -2.0 * pts.T
            inp[4, :r] = (pts * pts).sum(1)
        inp[3, :R] = 1.0
        inp[4, r:R] = BIG
        inp[0:3, R:R + M] = t[b].T
        inp[3, R:R + M] = (t[b] * t[b]).sum(1)
        if m_pad > M:               # padding cols must never win a row-min
            inp[3, R + M:] = BIG
        inp[4, R:] = 1.0
        in_maps.append({"inp": inp})

    res = run_bass_kernel_spmd(nc, in_maps, core_ids=list(range(n_cores)))

    per_sample = np.zeros(B, dtype=np.float64)
    for b in range(B):
        d1_sum = 0.0
        colmin = np.full(M, np.float32(BIG), dtype=np.float32)
        for h in range(cps):
            core = b * cps + h
            r = parts[core][1].shape[0]
            out = res.results[core]
            if r > 0:
                rowmin = out["rowmin"].T.ravel()[:r]       # n = i*128 + p
                d1_sum += np.sqrt(np.maximum(rowmin, 0.0)).sum(dtype=np.float64)
            colmin = np.minimum(colmin, -out["colmax"][0, :M])
        nv = int(mask[b].sum())
        cnt = max(nv, 1)
        m1 = d1_sum / cnt
        if nv == 0:
            colmin[:] = BIG        # reference: all rows masked -> d = BIG
        m2 = np.sqrt(np.maximum(colmin, 0.0)).mean(dtype=np.float64)
        per_sample[b] = 0.5 * (m1 + m2)

    return np.float32(per_sample.mean())


# revision 15
# speedup vs baseline: 1.2453x; 1.1220x over previous
"""Chamfer loss (ChamferDistanceL1-style) Trainium2 Bass kernel.

Problem: B=4 samples, N=M=4096 points, 3D. loss = mean_b 0.5*(m1_b + m2_b)
  m1 = masked mean over valid pred points of sqrt(min_m d[n,m])
  m2 = mean over target points of sqrt(min over *valid* n of d[n,m])
  d[n,m] = max(|p_n|^2 + |t_m|^2 - 2 p.t, 0)

Strategy (8 NeuronCores):
  - Host compacts each sample's pred points to the valid (label==1) subset
    (~halves the work), splits them across 2 cores -> 8 cores = 4 samples x 2.
  - Distances are produced by a single K=5 fp32 matmul per tile:
      lhsT col n = [-2px, -2py, -2pz, 1, |p_n|^2 (+BIG if padding)]
      rhs  col m = [ tx,   ty,   tz,  |t_m|^2, 1]
    so PSUM holds d[n,m] directly (before the max(.,0) clamp).
  - Per PSUM chunk [128, 2048]:
      DVE: free-axis reduce-min (exact fp32 row mins)
      ACT: negated fp16 copy PSUM -> SBUF (idle engine, off critical path)
      DVE: fp16 tensor_tensor max into the negated column accumulator
           (fp16 SBUF step-1 -> 2x DVE mode, halves the column-pass cost)
  - GPSIMD finishes each chunk's 128-way partition max (overlapped with the
    next chunk's compute); host does the final clamp/sqrt/means (tiny).
  - fp16 only touches the d2 (target->pred) path as a value rounding of the
    already-exact fp32 distances: |err| <= 2^-11 relative, ~1e-4 on the loss.
"""

import numpy as np

import concourse.bacc as bacc
import concourse.bass_isa as bass_isa
import concourse.tile as tile
from concourse import mybir
from concourse.bass_utils import run_bass_kernel_spmd

F32 = mybir.dt.float32
F16 = mybir.dt.float16
BIG = np.float32(1e10)  # matches the reference's masking constant
_NC_CACHE = {}

_P = 128          # partitions / rows per weight tile
_MM_FREE = 512    # fp32 matmul moving-dim limit (one PSUM bank)
_CHUNK = 2048     # PSUM chunk (4 banks); 2 bufs = all 8 banks


def _build_nc(r_tiles: int, m_pad: int):
    """Build + finalize the per-core Bass program for R=128*r_tiles pred rows
    and m_pad (multiple of _CHUNK) target columns."""
    R = r_tiles * _P
    n_chunks = m_pad // _CHUNK

    nc = bacc.Bacc("TRN2", target_bir_lowering=False)
    inp = nc.dram_tensor("inp", [5, R + m_pad], F32, kind="ExternalInput")
    rowmin_d = nc.dram_tensor("rowmin", [_P, r_tiles], F32, kind="ExternalOutput")
    colmax_d = nc.dram_tensor("colmax", [1, m_pad], F32, kind="ExternalOutput")
    warm_d = nc.dram_tensor("warm", [_P, 1], F32, kind="ExternalOutput")

    with tile.TileContext(nc) as tc:
        with tc.tile_pool(name="io", bufs=1) as io, \
             tc.tile_pool(name="ps", bufs=2, space="PSUM") as psp:
            # PE warmup: ~5 dummy matmuls during the input DMA keep the HAM
            # clock-gate busy so real matmuls run at full clock from the start.
            wsrc = io.tile([5, _MM_FREE], F32)
            nc.vector.memset(wsrc[:], 0.0)
            wps = psp.tile([_P, _MM_FREE], F32, tag="ps")
            for _ in range(5):
                nc.tensor.matmul(wps[:], wsrc[:, 0:_P], wsrc[:],
                                 start=True, stop=True)
            warm_sb = io.tile([_P, 1], F32)
            nc.vector.tensor_reduce(warm_sb[:], wps[:],
                                    axis=mybir.AxisListType.X,
                                    op=mybir.AluOpType.max)
            nc.sync.dma_start(out=warm_d[:, :], in_=warm_sb[:])

            # weights first (small), then one DMA per rhs chunk: the first
            # matmuls only wait on their own chunk's DMA.
            in_sb = io.tile([5, R + m_pad], F32)
            nc.sync.dma_start(out=in_sb[:, :R], in_=inp[:, :R])
            for c in range(n_chunks):
                cs = slice(R + c * _CHUNK, R + (c + 1) * _CHUNK)
                nc.sync.dma_start(out=in_sb[:, cs], in_=inp[:, cs])

            # negated fp16 column accumulator: holds max(-d) = -min(d)
            colacc = io.tile([_P, m_pad], F16)
            nc.any.memset(colacc[:], -60000.0)
            colred = io.tile([_P, m_pad], F32)

            rowstage = io.tile([_P, r_tiles * n_chunks], F32)

            with tc.tile_pool(name="scr", bufs=3) as scrp:
                for c in range(n_chunks):
                    for i in range(r_tiles):
                        lhsT = in_sb[:, i * _P:(i + 1) * _P]
                        ps = psp.tile([_P, _CHUNK], F32, tag="ps")
                        for s in range(_CHUNK // _MM_FREE):
                            col0 = R + c * _CHUNK + s * _MM_FREE
                            nc.tensor.matmul(
                                ps[:, s * _MM_FREE:(s + 1) * _MM_FREE],
                                lhsT,
                                in_sb[:, col0:col0 + _MM_FREE],
                                start=True, stop=True,
                            )
                        # ACT: scr = -d in fp16; frees the PSUM slot fast so
                        # the PE never stalls. Both reductions read scr.
                        scr = scrp.tile([_P, _CHUNK], F16, tag="scr")
                        nc.scalar.mul(scr[:], ps[:], -1.0)
                        k = i * n_chunks + c
                        nc.vector.tensor_reduce(
                            rowstage[:, k:k + 1], scr[:],
                            axis=mybir.AxisListType.X, op=mybir.AluOpType.max,
                        )
                        cs = slice(c * _CHUNK, (c + 1) * _CHUNK)
                        nc.vector.tensor_tensor(
                            out=colacc[:, cs], in0=scr[:], in1=colacc[:, cs],
                            op=mybir.AluOpType.max,
                        )
                    # chunk done: 128-way partition max on GPSIMD (overlaps
                    # the next chunk's matmuls/DVE work)
                    cs = slice(c * _CHUNK, (c + 1) * _CHUNK)
                    nc.gpsimd.partition_all_reduce(
                        colred[:, cs], colacc[:, cs],
                        channels=_P, reduce_op=bass_isa.ReduceOp.max,
                    )
                    nc.sync.dma_start(out=colmax_d[:, cs], in_=colred[0:1, cs])

            # rowstage holds max(-d); combine chunks, host negates.
            rowmin_sb = io.tile([_P, r_tiles], F32)
            nc.vector.tensor_reduce(
                rowmin_sb[:],
                rowstage[:].rearrange("p (i c) -> p i c", c=n_chunks),
                axis=mybir.AxisListType.X, op=mybir.AluOpType.max,
            )
            nc.sync.dma_start(out=rowmin_d[:, :], in_=rowmin_sb[:])
    nc.finalize()
    return nc


def _get_nc(r_tiles: int, m_pad: int):
    key = (r_tiles, m_pad)
    if key not in _NC_CACHE:
        _NC_CACHE[key] = _build_nc(r_tiles, m_pad)
    return _NC_CACHE[key]


def _chamfer_numpy(p, t, mask):
    """Blocked numpy fallback (exact), for odd configurations."""
    B = p.shape[0]
    per_sample = np.zeros(B, dtype=np.float64)
    for b in range(B):
        pb, tb = p[b], t[b]
        tn = (tb * tb).sum(1)
        pn = (pb * pb).sum(1)
        rowmin = np.full(pb.shape[0], np.inf, dtype=np.float32)
        colmin = np.full(tb.shape[0], np.float32(BIG), dtype=np.float32)
        step = 512
        for i in range(0, pb.shape[0], step):
            d = (pn[i:i + step, None] + tn[None, :]
                 - 2.0 * (pb[i:i + step] @ tb.T)).astype(np.float32)
            d = np.maximum(d, 0.0)
            rowmin[i:i + step] = d.min(axis=1)
            mrows = mask[b, i:i + step]
            if mrows.any():
                colmin = np.minimum(colmin, d[mrows].min(axis=0))
        cnt = max(int(mask[b].sum()), 1)
        m1 = np.sqrt(rowmin[mask[b]]).sum() / cnt
        m2 = np.sqrt(colmin).mean()
        per_sample[b] = 0.5 * (m1 + m2)
    return np.float32(per_sample.mean())


def kernel(pred_pc, target, label, nums, dense_nums):
    B = int(np.asarray(nums).shape[0])
    p = np.ascontiguousarray(np.asarray(pred_pc, dtype=np.float32)).reshape(B, -1, 3)
    t = np.ascontiguousarray(np.asarray(target, dtype=np.float32)).reshape(B, -1, 3)
    N = p.shape[1]
    M = t.shape[1]
    mask = (np.asarray(label).reshape(B, N) == 1)

    if B < 1 or B > 8 or M < 1:
        return _chamfer_numpy(p, t, mask)

    cps = max(1, 8 // B)          # cores per sample
    n_cores = B * cps
    m_pad = ((M + _CHUNK - 1) // _CHUNK) * _CHUNK

    # Split each sample's valid pred points across its cores.
    parts = []                    # (sample, pts[r,3]) per core
    for b in range(B):
        pv = p[b][mask[b]]
        for chunk in np.array_split(pv, cps, axis=0):
            parts.append((b, np.ascontiguousarray(chunk)))
    rmax = max(c.shape[0] for _, c in parts)
    R = max(_P, ((rmax + _P - 1) // _P) * _P)
    r_tiles = R // _P

    nc = _get_nc(r_tiles, m_pad)

    in_maps = []
    for b, pts in parts:
        r = pts.shape[0]
        inp = np.zeros((5, R + m_pad), dtype=np.float32)
        if r > 0:
            inp[0:3, :r] = -2.0 * pts.T
            inp[4, :r] = (pts * pts).sum(1)
        inp[3, :R] = 1.0
        inp[4, r:R] = BIG
        inp[0:3, R:R + M] = t[b].T
        inp[3, R:R + M] = (t[b] * t[b]).sum(1)
        if m_pad > M:               # padding cols must never win a row-min
            inp[3, R + M:] = BIG
        inp[4, R:] = 1.0
        in_maps.append({"inp": inp})

    res = run_bass_kernel_spmd(nc, in_maps, core_ids=list(range(n_cores)))

    per_sample = np.zeros(B, dtype=np.float64)
    for b in range(B):
        d1_sum = 0.0
        colmin = np.full(M, np.float32(BIG), dtype=np.float32)
        for h in range(cps):
            core = b * cps + h
            r = parts[core][1].shape[0]
            out = res.results[core]
            if r > 0:
                rowmin = -out["rowmin"].T.ravel()[:r]      # n = i*128 + p
                d1_sum += np.sqrt(np.maximum(rowmin, 0.0)).sum(dtype=np.float64)
            colmin = np.minimum(colmin, -out["colmax"][0, :M])
        nv = int(mask[b].sum())
        cnt = max(nv, 1)
        m1 = d1_sum / cnt
        if nv == 0:
            colmin[:] = BIG        # reference: all rows masked -> d = BIG
        m2 = np.sqrt(np.maximum(colmin, 0.0)).mean(dtype=np.float64)
        per_sample[b] = 0.5 * (m1 + m2)

    return np.float32(per_sample.mean())


# revision 16
# speedup vs baseline: 1.3486x; 1.0830x over previous
"""Chamfer loss (ChamferDistanceL1-style) Trainium2 Bass kernel.

Problem: B=4 samples, N=M=4096 points, 3D. loss = mean_b 0.5*(m1_b + m2_b)
  m1 = masked mean over valid pred points of sqrt(min_m d[n,m])
  m2 = mean over target points of sqrt(min over *valid* n of d[n,m])
  d[n,m] = max(|p_n|^2 + |t_m|^2 - 2 p.t, 0)

Strategy (8 NeuronCores):
  - Host compacts each sample's pred points to the valid (label==1) subset
    (~halves the work), splits them across 2 cores -> 8 cores = 4 samples x 2.
  - Distances are produced by a single K=5 fp32 matmul per tile:
      lhsT col n = [-2px, -2py, -2pz, 1, |p_n|^2 (+BIG if padding)]
      rhs  col m = [ tx,   ty,   tz,  |t_m|^2, 1]
    so PSUM holds d[n,m] directly (before the max(.,0) clamp).
  - Per PSUM chunk [128, 2048]:
      DVE: free-axis reduce-min (exact fp32 row mins)
      ACT: negated fp16 copy PSUM -> SBUF (idle engine, off critical path)
      DVE: fp16 tensor_tensor max into the negated column accumulator
           (fp16 SBUF step-1 -> 2x DVE mode, halves the column-pass cost)
  - GPSIMD finishes each chunk's 128-way partition max (overlapped with the
    next chunk's compute); host does the final clamp/sqrt/means (tiny).
  - fp16 only touches the d2 (target->pred) path as a value rounding of the
    already-exact fp32 distances: |err| <= 2^-11 relative, ~1e-4 on the loss.
"""

import numpy as np

import concourse.bacc as bacc
import concourse.bass_isa as bass_isa
import concourse.tile as tile
from concourse import mybir
from concourse.bass_utils import run_bass_kernel_spmd

F32 = mybir.dt.float32
F16 = mybir.dt.float16
BIG = np.float32(1e10)  # matches the reference's masking constant
_NC_CACHE = {}

_P = 128          # partitions / rows per weight tile
_MM_FREE = 512    # fp32 matmul moving-dim limit (one PSUM bank)
_CHUNK = 2048     # PSUM chunk (4 banks); 2 bufs = all 8 banks


def _build_nc(r_tiles: int, m_pad: int):
    """Build + finalize the per-core Bass program for R=128*r_tiles pred rows
    and m_pad (multiple of _CHUNK) target columns."""
    R = r_tiles * _P
    n_chunks = m_pad // _CHUNK

    nc = bacc.Bacc("TRN2", target_bir_lowering=False)
    inp = nc.dram_tensor("inp", [5, R + m_pad], F32, kind="ExternalInput")
    rowmin_d = nc.dram_tensor("rowmin", [_P, r_tiles], F32, kind="ExternalOutput")
    colmax_d = nc.dram_tensor("colmax", [1, m_pad], F32, kind="ExternalOutput")
    warm_d = nc.dram_tensor("warm", [_P, 1], F32, kind="ExternalOutput")

    with tile.TileContext(nc) as tc:
        with tc.tile_pool(name="io", bufs=1) as io, \
             tc.tile_pool(name="ps", bufs=2, space="PSUM") as psp:
            # PE warmup: a dummy matmul during the input DMA starts the HAM
            # clock-gate ramp so real matmuls run closer to full clock.
            wsrc = io.tile([5, _MM_FREE], F32)
            nc.vector.memset(wsrc[:], 0.0)
            wps = psp.tile([_P, _MM_FREE], F32, tag="ps")
            nc.tensor.matmul(wps[:], wsrc[:, 0:_P], wsrc[:],
                             start=True, stop=True)
            warm_sb = io.tile([_P, 1], F32)
            nc.vector.tensor_reduce(warm_sb[:], wps[:],
                                    axis=mybir.AxisListType.X,
                                    op=mybir.AluOpType.max)
            nc.sync.dma_start(out=warm_d[:, :], in_=warm_sb[:])

            # weights first (small), then one DMA per rhs chunk: the first
            # matmuls only wait on their own chunk's DMA.
            in_sb = io.tile([5, R + m_pad], F32)
            nc.sync.dma_start(out=in_sb[:, :R], in_=inp[:, :R])
            for c in range(n_chunks):
                cs = slice(R + c * _CHUNK, R + (c + 1) * _CHUNK)
                nc.sync.dma_start(out=in_sb[:, cs], in_=inp[:, cs])

            # negated fp16 column accumulator: holds max(-d) = -min(d)
            colacc = io.tile([_P, m_pad], F16)
            nc.any.memset(colacc[:], -60000.0)
            colred = io.tile([_P, m_pad], F32)

            rowstage = io.tile([_P, r_tiles * n_chunks], F32)

            with tc.tile_pool(name="scr", bufs=3) as scrp:
                for c in range(n_chunks):
                    for i in range(r_tiles):
                        lhsT = in_sb[:, i * _P:(i + 1) * _P]
                        ps = psp.tile([_P, _CHUNK], F32, tag="ps")
                        for s in range(_CHUNK // _MM_FREE):
                            col0 = R + c * _CHUNK + s * _MM_FREE
                            nc.tensor.matmul(
                                ps[:, s * _MM_FREE:(s + 1) * _MM_FREE],
                                lhsT,
                                in_sb[:, col0:col0 + _MM_FREE],
                                start=True, stop=True,
                            )
                        # ACT: scr = -d in fp16; frees the PSUM slot fast so
                        # the PE never stalls. Both reductions read scr.
                        scr = scrp.tile([_P, _CHUNK], F16, tag="scr")
                        nc.scalar.mul(scr[:], ps[:], -1.0)
                        k = i * n_chunks + c
                        nc.vector.tensor_reduce(
                            rowstage[:, k:k + 1], scr[:],
                            axis=mybir.AxisListType.X, op=mybir.AluOpType.max,
                        )
                        cs = slice(c * _CHUNK, (c + 1) * _CHUNK)
                        nc.vector.tensor_tensor(
                            out=colacc[:, cs], in0=scr[:], in1=colacc[:, cs],
                            op=mybir.AluOpType.max,
                        )
                    # chunk done: 128-way partition max on GPSIMD (overlaps
                    # the next chunk's matmuls/DVE work)
                    cs = slice(c * _CHUNK, (c + 1) * _CHUNK)
                    nc.gpsimd.partition_all_reduce(
                        colred[:, cs], colacc[:, cs],
                        channels=_P, reduce_op=bass_isa.ReduceOp.max,
                    )
                    nc.sync.dma_start(out=colmax_d[:, cs], in_=colred[0:1, cs])

            # rowstage holds max(-d); combine chunks, host negates.
            rowmin_sb = io.tile([_P, r_tiles], F32)
            nc.vector.tensor_reduce(
                rowmin_sb[:],
                rowstage[:].rearrange("p (i c) -> p i c", c=n_chunks),
                axis=mybir.AxisListType.X, op=mybir.AluOpType.max,
            )
            nc.sync.dma_start(out=rowmin_d[:, :], in_=rowmin_sb[:])
    nc.finalize()
    return nc


def _get_nc(r_tiles: int, m_pad: int):
    key = (r_tiles, m_pad)
    if key not in _NC_CACHE:
        _NC_CACHE[key] = _build_nc(r_tiles, m_pad)
    return _NC_CACHE[key]


def _chamfer_numpy(p, t, mask):
    """Blocked numpy fallback (exact), for odd configurations."""
    B = p.shape[0]
    per_sample = np.zeros(B, dtype=np.float64)
    for b in range(B):
        pb, tb = p[b], t[b]
        tn = (tb * tb).sum(1)
        pn = (pb * pb).sum(1)
        rowmin = np.full(pb.shape[0], np.inf, dtype=np.float32)
        colmin = np.full(tb.shape[0], np.float32(BIG), dtype=np.float32)
        step = 512
        for i in range(0, pb.shape[0], step):
            d = (pn[i:i + step, None] + tn[None, :]
                 - 2.0 * (pb[i:i + step] @ tb.T)).astype(np.float32)
            d = np.maximum(d, 0.0)
            rowmin[i:i + step] = d.min(axis=1)
            mrows = mask[b, i:i + step]
            if mrows.any():
                colmin = np.minimum(colmin, d[mrows].min(axis=0))
        cnt = max(int(mask[b].sum()), 1)
        m1 = np.sqrt(rowmin[mask[b]]).sum() / cnt
        m2 = np.sqrt(colmin).mean()
        per_sample[b] = 0.5 * (m1 + m2)
    return np.float32(per_sample.mean())


def kernel(pred_pc, target, label, nums, dense_nums):
    B = int(np.asarray(nums).shape[0])
    p = np.ascontiguousarray(np.asarray(pred_pc, dtype=np.float32)).reshape(B, -1, 3)
    t = np.ascontiguousarray(np.asarray(target, dtype=np.float32)).reshape(B, -1, 3)
    N = p.shape[1]
    M = t.shape[1]
    mask = (np.asarray(label).reshape(B, N) == 1)

    if B < 1 or B > 8 or M < 1:
        return _chamfer_numpy(p, t, mask)

    cps = max(1, 8 // B)          # cores per sample
    n_cores = B * cps
    m_pad = ((M + _CHUNK - 1) // _CHUNK) * _CHUNK

    # Split each sample's valid pred points across its cores.
    parts = []                    # (sample, pts[r,3]) per core
    for b in range(B):
        pv = p[b][mask[b]]
        for chunk in np.array_split(pv, cps, axis=0):
            parts.append((b, np.ascontiguousarray(chunk)))
    rmax = max(c.shape[0] for _, c in parts)
    R = max(_P, ((rmax + _P - 1) // _P) * _P)
    r_tiles = R // _P

    nc = _get_nc(r_tiles, m_pad)

    in_maps = []
    for b, pts in parts:
        r = pts.shape[0]
        inp = np.zeros((5, R + m_pad), dtype=np.float32)
        if r > 0:
            inp[0:3, :r] = -2.0 * pts.T
            inp[4, :r] = (pts * pts).sum(1)
        inp[3, :R] = 1.0
        inp[4, r:R] = BIG
        inp[0:3, R:R + M] = t[b].T
        inp[3, R:R + M] = (t[b] * t[b]).sum(1)
        if m_pad > M:               # padding cols must never win a row-min
            inp[3, R + M:] = BIG
        inp[4, R:] = 1.0
        in_maps.append({"inp": inp})

    res = run_bass_kernel_spmd(nc, in_maps, core_ids=list(range(n_cores)))

    per_sample = np.zeros(B, dtype=np.float64)
    for b in range(B):
        d1_sum = 0.0
        colmin = np.full(M, np.float32(BIG), dtype=np.float32)
        for h in range(cps):
            core = b * cps + h
            r = parts[core][1].shape[0]
            out = res.results[core]
            if r > 0:
                rowmin = -out["rowmin"].T.ravel()[:r]      # n = i*128 + p
                d1_sum += np.sqrt(np.maximum(rowmin, 0.0)).sum(dtype=np.float64)
            colmin = np.minimum(colmin, -out["colmax"][0, :M])
        nv = int(mask[b].sum())
        cnt = max(nv, 1)
        m1 = d1_sum / cnt
        if nv == 0:
            colmin[:] = BIG        # reference: all rows masked -> d = BIG
        m2 = np.sqrt(np.maximum(colmin, 0.0)).mean(dtype=np.float64)
        per_sample[b] = 0.5 * (m1 + m2)

    return np.float32(per_sample.mean())


# revision 19
# speedup vs baseline: 1.5156x; 1.1238x over previous
"""Chamfer loss (ChamferDistanceL1-style) Trainium2 Bass kernel.

Problem: B=4 samples, N=M=4096 points, 3D. loss = mean_b 0.5*(m1_b + m2_b)
  m1 = masked mean over valid pred points of sqrt(min_m d[n,m])
  m2 = mean over target points of sqrt(min over *valid* n of d[n,m])
  d[n,m] = max(|p_n|^2 + |t_m|^2 - 2 p.t, 0)

Strategy (8 NeuronCores):
  - Host compacts each sample's pred points to the valid (label==1) subset
    (~halves the work), splits them across 2 cores -> 8 cores = 4 samples x 2.
  - Distances are produced by a single K=5 fp32 matmul per tile:
      lhsT col n = [-2px, -2py, -2pz, 1, |p_n|^2 (+BIG if padding)]
      rhs  col m = [ tx,   ty,   tz,  |t_m|^2, 1]
    so PSUM holds d[n,m] directly (before the max(.,0) clamp).
  - Per PSUM chunk [128, 2048]:
      DVE: free-axis reduce-min (exact fp32 row mins)
      ACT: negated fp16 copy PSUM -> SBUF (idle engine, off critical path)
      DVE: fp16 tensor_tensor max into the negated column accumulator
           (fp16 SBUF step-1 -> 2x DVE mode, halves the column-pass cost)
  - GPSIMD finishes each chunk's 128-way partition max (overlapped with the
    next chunk's compute); host does the final clamp/sqrt/means (tiny).
  - fp16 only touches the d2 (target->pred) path as a value rounding of the
    already-exact fp32 distances: |err| <= 2^-11 relative, ~1e-4 on the loss.
"""

import numpy as np

import concourse.bacc as bacc
import concourse.bass_isa as bass_isa
import concourse.tile as tile
from concourse import mybir
from concourse.bass_utils import run_bass_kernel_spmd

F32 = mybir.dt.float32
F16 = mybir.dt.float16
BIG = np.float32(1e10)  # matches the reference's masking constant
_NC_CACHE = {}

_P = 128          # partitions / rows per weight tile
_MM_FREE = 512    # fp32 matmul moving-dim limit (one PSUM bank)
_CHUNK = 2048     # PSUM chunk (4 banks); 2 bufs = all 8 banks


def _build_nc(r_tiles: int, m_pad: int):
    """Build + finalize the per-core Bass program for R=128*r_tiles pred rows
    and m_pad (multiple of _CHUNK) target columns."""
    R = r_tiles * _P
    n_chunks = m_pad // _CHUNK

    nc = bacc.Bacc("TRN2", target_bir_lowering=False)
    inp = nc.dram_tensor("inp", [5, R + m_pad], F32, kind="ExternalInput")
    rowmin_d = nc.dram_tensor("rowmin", [_P, r_tiles], F32, kind="ExternalOutput")
    colmax_d = nc.dram_tensor("colmax", [1, m_pad], F32, kind="ExternalOutput")
    warm_d = nc.dram_tensor("warm", [_P, 1], F32, kind="ExternalOutput")

    with tile.TileContext(nc) as tc:
        with tc.tile_pool(name="io", bufs=1) as io, \
             tc.tile_pool(name="ps", bufs=2, space="PSUM") as psp:
            # PE warmup: a dummy matmul during the input DMA starts the HAM
            # clock-gate ramp so real matmuls run closer to full clock.
            wsrc = io.tile([5, _MM_FREE], F32)
            nc.vector.memset(wsrc[:], 0.0)
            wps = psp.tile([_P, _MM_FREE], F32, tag="ps")
            nc.tensor.matmul(wps[:], wsrc[:, 0:_P], wsrc[:],
                             start=True, stop=True)
            warm_sb = io.tile([_P, 1], F32)
            nc.vector.tensor_reduce(warm_sb[:], wps[:],
                                    axis=mybir.AxisListType.X,
                                    op=mybir.AluOpType.max)
            nc.sync.dma_start(out=warm_d[:, :], in_=warm_sb[:])

            # weights first (small), then one DMA per rhs chunk: the first
            # matmuls only wait on their own chunk's DMA.
            in_sb = io.tile([5, R + m_pad], F32)
            nc.sync.dma_start(out=in_sb[:, :R], in_=inp[:, :R])
            for c in range(n_chunks):
                cs = slice(R + c * _CHUNK, R + (c + 1) * _CHUNK)
                nc.sync.dma_start(out=in_sb[:, cs], in_=inp[:, cs])

            # negated fp16 column accumulator: holds max(-d) = -min(d)
            colacc = io.tile([_P, m_pad], F16)
            nc.any.memset(colacc[:], -60000.0)
            colred = io.tile([_P, m_pad], F32)

            rowstage = io.tile([_P, r_tiles * n_chunks], F32)

            with tc.tile_pool(name="scr", bufs=3) as scrp:
                for c in range(n_chunks):
                    for i in range(r_tiles):
                        lhsT = in_sb[:, i * _P:(i + 1) * _P]
                        ps = psp.tile([_P, _CHUNK], F32, tag="ps")
                        for s in range(_CHUNK // _MM_FREE):
                            col0 = R + c * _CHUNK + s * _MM_FREE
                            nc.tensor.matmul(
                                ps[:, s * _MM_FREE:(s + 1) * _MM_FREE],
                                lhsT,
                                in_sb[:, col0:col0 + _MM_FREE],
                                start=True, stop=True,
                            )
                        # ACT: scr = -d in fp16; frees the PSUM slot fast so
                        # the PE never stalls. Both reductions read scr.
                        scr = scrp.tile([_P, _CHUNK], F16, tag="scr")
                        nc.scalar.mul(scr[:], ps[:], -1.0)
                        # row max(-d): two fp16 TT-max tree levels run in the
                        # DVE 2x mode before the (1x-only) tensor_reduce.
                        h1 = _CHUNK // 2
                        s1 = scrp.tile([_P, h1], F16, tag="s1")
                        nc.vector.tensor_tensor(out=s1[:], in0=scr[:, :h1],
                                                in1=scr[:, h1:],
                                                op=mybir.AluOpType.max)
                        h2 = h1 // 2
                        s2 = scrp.tile([_P, h2], F16, tag="s2")
                        nc.vector.tensor_tensor(out=s2[:], in0=s1[:, :h2],
                                                in1=s1[:, h2:],
                                                op=mybir.AluOpType.max)
                        k = i * n_chunks + c
                        nc.vector.tensor_reduce(
                            rowstage[:, k:k + 1], s2[:],
                            axis=mybir.AxisListType.X, op=mybir.AluOpType.max,
                        )
                        cs = slice(c * _CHUNK, (c + 1) * _CHUNK)
                        nc.vector.tensor_tensor(
                            out=colacc[:, cs], in0=scr[:], in1=colacc[:, cs],
                            op=mybir.AluOpType.max,
                        )
                    # chunk done: 128-way partition max on GPSIMD (overlaps
                    # the next chunk's matmuls/DVE work)
                    cs = slice(c * _CHUNK, (c + 1) * _CHUNK)
                    nc.gpsimd.partition_all_reduce(
                        colred[:, cs], colacc[:, cs],
                        channels=_P, reduce_op=bass_isa.ReduceOp.max,
                    )
                    nc.sync.dma_start(out=colmax_d[:, cs], in_=colred[0:1, cs])

            # rowstage holds max(-d); combine chunks, host negates.
            rowmin_sb = io.tile([_P, r_tiles], F32)
            nc.vector.tensor_reduce(
                rowmin_sb[:],
                rowstage[:].rearrange("p (i c) -> p i c", c=n_chunks),
                axis=mybir.AxisListType.X, op=mybir.AluOpType.max,
            )
            nc.sync.dma_start(out=rowmin_d[:, :], in_=rowmin_sb[:])
    nc.finalize()
    return nc


def _get_nc(r_tiles: int, m_pad: int):
    key = (r_tiles, m_pad)
    if key not in _NC_CACHE:
        _NC_CACHE[key] = _build_nc(r_tiles, m_pad)
    return _NC_CACHE[key]


def _chamfer_numpy(p, t, mask):
    """Blocked numpy fallback (exact), for odd configurations."""
    B = p.shape[0]
    per_sample = np.zeros(B, dtype=np.float64)
    for b in range(B):
        pb, tb = p[b], t[b]
        tn = (tb * tb).sum(1)
        pn = (pb * pb).sum(1)
        rowmin = np.full(pb.shape[0], np.inf, dtype=np.float32)
        colmin = np.full(tb.shape[0], np.float32(BIG), dtype=np.float32)
        step = 512
        for i in range(0, pb.shape[0], step):
            d = (pn[i:i + step, None] + tn[None, :]
                 - 2.0 * (pb[i:i + step] @ tb.T)).astype(np.float32)
            d = np.maximum(d, 0.0)
            rowmin[i:i + step] = d.min(axis=1)
            mrows = mask[b, i:i + step]
            if mrows.any():
                colmin = np.minimum(colmin, d[mrows].min(axis=0))
        cnt = max(int(mask[b].sum()), 1)
        m1 = np.sqrt(rowmin[mask[b]]).sum() / cnt
        m2 = np.sqrt(colmin).mean()
        per_sample[b] = 0.5 * (m1 + m2)
    return np.float32(per_sample.mean())


def kernel(pred_pc, target, label, nums, dense_nums):
    B = int(np.asarray(nums).shape[0])
    p = np.ascontiguousarray(np.asarray(pred_pc, dtype=np.float32)).reshape(B, -1, 3)
    t = np.ascontiguousarray(np.asarray(target, dtype=np.float32)).reshape(B, -1, 3)
    N = p.shape[1]
    M = t.shape[1]
    mask = (np.asarray(label).reshape(B, N) == 1)

    if B < 1 or B > 8 or M < 1:
        return _chamfer_numpy(p, t, mask)

    cps = max(1, 8 // B)          # cores per sample
    n_cores = B * cps
    m_pad = ((M + _CHUNK - 1) // _CHUNK) * _CHUNK

    # Split each sample's valid pred points across its cores.
    parts = []                    # (sample, pts[r,3]) per core
    for b in range(B):
        pv = p[b][mask[b]]
        for chunk in np.array_split(pv, cps, axis=0):
            parts.append((b, np.ascontiguousarray(chunk)))
    rmax = max(c.shape[0] for _, c in parts)
    # Rows past a full 128-tile boundary would cost a whole extra matmul
    # pass; when that overflow is small, handle those rows on the host.
    r_floor = max(_P, (rmax // _P) * _P)
    if 0 < rmax - r_floor <= 48:
        R = r_floor
    else:
        R = max(_P, ((rmax + _P - 1) // _P) * _P)
    r_tiles = R // _P

    nc = _get_nc(r_tiles, m_pad)

    in_maps = []
    for b, pts in parts:
        r = min(pts.shape[0], R)
        inp = np.zeros((5, R + m_pad), dtype=np.float32)
        if r > 0:
            inp[0:3, :r] = -2.0 * pts[:r].T
            inp[4, :r] = (pts[:r] * pts[:r]).sum(1)
        inp[3, :R] = 1.0
        inp[4, r:R] = BIG
        inp[0:3, R:R + M] = t[b].T
        inp[3, R:R + M] = (t[b] * t[b]).sum(1)
        if m_pad > M:               # padding cols must never win a row-min
            inp[3, R + M:] = BIG
        inp[4, R:] = 1.0
        in_maps.append({"inp": inp})

    res = run_bass_kernel_spmd(nc, in_maps, core_ids=list(range(n_cores)))

    per_sample = np.zeros(B, dtype=np.float64)
    for b in range(B):
        d1_sum = 0.0
        colmin = np.full(M, np.float32(BIG), dtype=np.float32)
        tn_b = None
        for h in range(cps):
            core = b * cps + h
            pts = parts[core][1]
            r = min(pts.shape[0], R)
            out = res.results[core]
            if r > 0:
                rowmin = -out["rowmin"].T.ravel()[:r]      # n = i*128 + p
                d1_sum += np.sqrt(np.maximum(rowmin, 0.0)).sum(dtype=np.float64)
            colmin = np.minimum(colmin, -out["colmax"][0, :M])
            if pts.shape[0] > R:                           # host overflow rows
                hp = pts[R:]
                if tn_b is None:
                    tn_b = (t[b] * t[b]).sum(1)
                d = ((hp * hp).sum(1)[:, None] + tn_b[None, :]
                     - 2.0 * (hp @ t[b].T)).astype(np.float32)
                d = np.maximum(d, 0.0)
                d1_sum += np.sqrt(d.min(axis=1)).sum(dtype=np.float64)
                colmin = np.minimum(colmin, d.min(axis=0))
        nv = int(mask[b].sum())
        cnt = max(nv, 1)
        m1 = d1_sum / cnt
        if nv == 0:
            colmin[:] = BIG        # reference: all rows masked -> d = BIG
        m2 = np.sqrt(np.maximum(colmin, 0.0)).mean(dtype=np.float64)
        per_sample[b] = 0.5 * (m1 + m2)

    return np.float32(per_sample.mean())
